# revision 3
# baseline (speedup 1.0000x reference)
"""Trainium2 Bass kernel for APConv GNN message passing (8 NeuronCores).

Strategy: shard edges by destination-AP range (1250 APs per core), so the
segment-sum is core-local and no collective is needed.  The host packs, per
core, a dst-sorted / transposed / bf16 message matrix msgT = [edge_hid ;
ue_hid[src]]^T and per-128-edge-tile slot columns.  The device streams msgT
tiles through mlp1's first linear (one matmul per tile), relu (ScalarE),
builds a one-hot segment matrix S from the slot column (VectorE is_equal
against a baked iota matrix), and accumulates r += relu1^T @ S into PSUM per
128-AP block.  The linear W1b is folded past the segment sum (it commutes),
so mlp1's second matmul and all of mlp2 run once per AP block instead of per
edge.  All cores execute one identical SPMD graph: per-block tile counts are
equalized across cores with zero-padded edges routed to an out-of-range slot.
"""
import sys

sys.path.insert(0, "/opt/trn_rl_repo")

import numpy as np
import ml_dtypes

import concourse.bacc as bacc
import concourse.mybir as mybir
from concourse import tile
from concourse.bass_utils import run_bass_kernel_spmd

BF16 = ml_dtypes.bfloat16
F32 = np.float32

N_UE, N_AP, N_E, D, H = 100000, 10000, 1600000, 64, 128
NCORE = 8
APC = N_AP // NCORE            # 1250 APs per core
NBLK = (APC + 127) // 128      # 10 blocks of 128 AP slots
CH = 8                         # msgT tiles per DMA chunk


def _preprocess(inputs):
    """Sort edges by dst, partition by (core, block), pad to equal tile
    counts, and build per-core device input arrays."""
    src = np.asarray(inputs["src"]).astype(np.int64)
    dst = np.asarray(inputs["dst"]).astype(np.int64)
    edge_hid = np.asarray(inputs["edge_hid"], dtype=F32)
    ue_hid = np.asarray(inputs["ue_hid"], dtype=F32)

    order = np.argsort(dst, kind="stable")
    dst_s = dst[order]
    core_s = dst_s // APC
    rem = dst_s % APC
    blk_s = rem // 128
    slot_s = (rem % 128).astype(F32)

    gid = core_s * NBLK + blk_s               # non-decreasing along sorted stream
    counts = np.bincount(gid, minlength=NCORE * NBLK).reshape(NCORE, NBLK)
    T_blk = [max(1, int(np.ceil(counts[:, b].max() / 128))) for b in range(NBLK)]
    NTILE = sum(T_blk)
    col0 = np.concatenate([[0], np.cumsum(np.array(T_blk) * 128)])
    gstart = np.concatenate([[0], np.cumsum(counts.reshape(-1))])

    per_core = []
    for c in range(NCORE):
        # edge-major packed [NTILE*128, 128] then transpose once
        msg = np.zeros((NTILE * 128, 2 * D), dtype=BF16)
        slots = np.full((NTILE * 128,), 999.0, dtype=F32)
        cnt = np.zeros((NBLK, 128), dtype=F32)
        for b in range(NBLK):
            g = c * NBLK + b
            s0, s1 = gstart[g], gstart[g + 1]
            idx = order[s0:s1]
            n = s1 - s0
            off = col0[b]
            msg[off:off + n, :D] = edge_hid[idx]
            msg[off:off + n, D:] = ue_hid[src[idx]]
            slots[off:off + n] = slot_s[s0:s1]
            cnt[b] = np.bincount(
                ((dst[idx] % APC) % 128).astype(np.int64), minlength=128
            )[:128]
        msgT = np.ascontiguousarray(msg.T)                      # [128, NTILE*128]
        slotsT = np.ascontiguousarray(slots.reshape(NTILE, 128).T)  # [128, NTILE]
        per_core.append({"msgT": msgT, "slots": slotsT, "cnt": cnt})
    return per_core, T_blk, NTILE


def _build_graph(T_blk, has_b1a, has_b1b):
    NBLK_ = len(T_blk)
    NTILE = sum(T_blk)
    W = NTILE * 128
    dt = mybir.dt
    nc = bacc.Bacc(None, target_bir_lowering=False)

    msgT_e = nc.declare_dram_parameter("msgT", [128, W], dt.bfloat16, isOutput=False)
    slots_e = nc.declare_dram_parameter("slots", [128, NTILE], dt.float32, isOutput=False)
    apT_e = nc.declare_dram_parameter("apT", [D, NBLK_ * 128], dt.bfloat16, isOutput=False)
    w1a_e = nc.declare_dram_parameter("w1a", [128, H], dt.bfloat16, isOutput=False)
    w1b_e = nc.declare_dram_parameter("w1b", [H, H], dt.bfloat16, isOutput=False)
    w2aap_e = nc.declare_dram_parameter("w2a_ap", [D, H], dt.bfloat16, isOutput=False)
    w2aag_e = nc.declare_dram_parameter("w2a_agg", [H, H], dt.bfloat16, isOutput=False)
    w2b_e = nc.declare_dram_parameter("w2b", [H, D], dt.bfloat16, isOutput=False)
    b2a_e = nc.declare_dram_parameter("b2a", [H, 1], dt.float32, isOutput=False)
    b2b_e = nc.declare_dram_parameter("b2b", [D, 1], dt.float32, isOutput=False)
    iota_e = nc.declare_dram_parameter("iota", [128, 128], dt.float32, isOutput=False)
    if has_b1a:
        b1am_e = nc.declare_dram_parameter("b1a_mat", [128, H], dt.float32, isOutput=False)
    if has_b1b:
        cb1b_e = nc.declare_dram_parameter("cntb1b", [H, NBLK_ * 128], dt.float32, isOutput=False)
    out_e = nc.declare_dram_parameter("out", [D, NBLK_ * 128], dt.float32, isOutput=True)

    AF = mybir.ActivationFunctionType
    with tile.TileContext(nc) as tc:
        with (
            tc.tile_pool(name="const", bufs=1) as cpool,
            tc.tile_pool(name="msg", bufs=4) as mpool,
            tc.tile_pool(name="work", bufs=4) as wpool,
            tc.tile_pool(name="blk", bufs=2) as bpool,
            tc.tile_pool(name="ph1", bufs=2, space="PSUM") as ph1,
            tc.tile_pool(name="pr", bufs=2, space="PSUM") as pr,
            tc.tile_pool(name="pblk", bufs=1, space="PSUM") as pblk,
        ):
            def ld(ext, shape, dtype, tag):
                t = cpool.tile(shape, dtype, tag=tag)
                nc.sync.dma_start(t[:], ext[:])
                return t

            w1a_t = ld(w1a_e, [128, H], dt.bfloat16, "w1a")
            w1b_t = ld(w1b_e, [H, H], dt.bfloat16, "w1b")
            w2aap_t = ld(w2aap_e, [D, H], dt.bfloat16, "w2aap")
            w2aag_t = ld(w2aag_e, [H, H], dt.bfloat16, "w2aag")
            w2b_t = ld(w2b_e, [H, D], dt.bfloat16, "w2b")
            b2a_t = ld(b2a_e, [H, 1], dt.float32, "b2a")
            b2b_t = ld(b2b_e, [D, 1], dt.float32, "b2b")
            iota_t = ld(iota_e, [128, 128], dt.float32, "iota")
            slots_t = ld(slots_e, [128, NTILE], dt.float32, "slots")
            apT_t = ld(apT_e, [D, NBLK_ * 128], dt.bfloat16, "apT")
            if has_b1a:
                b1am_t = ld(b1am_e, [128, H], dt.float32, "b1am")
            if has_b1b:
                cb1b_t = ld(cb1b_e, [H, NBLK_ * 128], dt.float32, "cb1b")

            g = 0
            for b in range(NBLK_):
                Tb = T_blk[b]
                r_ps = pr.tile([H, 128], dt.float32, tag="r")
                chunk = None
                for ti in range(Tb):
                    j = ti % CH
                    if j == 0:
                        wch = min(CH, Tb - ti)
                        chunk = mpool.tile([128, CH * 128], dt.bfloat16, tag="chunk")
                        nc.sync.dma_start(
                            chunk[:, : wch * 128],
                            msgT_e[:, (g) * 128 : (g + wch) * 128],
                        )
                    msg_t = chunk[:, j * 128 : (j + 1) * 128]
                    h1 = ph1.tile([128, H], dt.float32, tag="h1")
                    nc.tensor.matmul(h1[:], msg_t, w1a_t[:], start=True, stop=True)
                    relu1 = wpool.tile([128, H], dt.bfloat16, tag="relu1")
                    if has_b1a:
                        tmp = wpool.tile([128, H], dt.float32, tag="h1b")
                        nc.vector.tensor_tensor(tmp[:], h1[:], b1am_t[:], mybir.AluOpType.add)
                        nc.scalar.activation(relu1[:], tmp[:], AF.Relu)
                    else:
                        nc.scalar.activation(relu1[:], h1[:], AF.Relu)
                    S = wpool.tile([128, 128], dt.bfloat16, tag="S")
                    nc.vector.tensor_scalar(
                        S[:], iota_t[:], slots_t[:, g : g + 1], None,
                        mybir.AluOpType.is_equal,
                    )
                    nc.tensor.matmul(
                        r_ps[:], relu1[:], S[:],
                        start=(ti == 0), stop=(ti == Tb - 1),
                    )
                    g += 1
                # ---- per-block epilogue ----
                rT = bpool.tile([H, 128], dt.bfloat16, tag="rT")
                nc.scalar.copy(rT[:], r_ps[:])
                agg_ps = pblk.tile([H, 128], dt.float32, tag="agg")
                nc.tensor.matmul(agg_ps[:], w1b_t[:], rT[:], start=True, stop=True)
                if has_b1b:
                    nc.vector.tensor_tensor(
                        agg_ps[:], agg_ps[:], cb1b_t[:, b * 128 : (b + 1) * 128],
                        mybir.AluOpType.add,
                    )
                aggT = bpool.tile([H, 128], dt.bfloat16, tag="aggT")
                nc.scalar.copy(aggT[:], agg_ps[:])
                z_ps = pblk.tile([H, 128], dt.float32, tag="z")
                nc.tensor.matmul(z_ps[:], w2aap_t[:], apT_t[:, b * 128 : (b + 1) * 128],
                                 start=True, stop=False)
                nc.tensor.matmul(z_ps[:], w2aag_t[:], aggT[:], start=False, stop=True)
                z1r = bpool.tile([H, 128], dt.bfloat16, tag="z1r")
                nc.scalar.activation(z1r[:], z_ps[:], AF.Relu, bias=b2a_t[:])
                o_ps = pblk.tile([D, 128], dt.float32, tag="o")
                nc.tensor.matmul(o_ps[:], w2b_t[:], z1r[:], start=True, stop=True)
                o_sb = bpool.tile([D, 128], dt.float32, tag="osb")
                nc.scalar.activation(o_sb[:], o_ps[:], AF.Identity, bias=b2b_t[:])
                nc.sync.dma_start(out_e[:, b * 128 : (b + 1) * 128], o_sb[:])
    nc.finalize()
    return nc


def _common_inputs(inputs, has_b1a):
    W1a = np.asarray(inputs["W1a"], dtype=F32)
    W1b = np.asarray(inputs["W1b"], dtype=F32)
    W2a = np.asarray(inputs["W2a"], dtype=F32)
    W2b = np.asarray(inputs["W2b"], dtype=F32)
    b2a = np.asarray(inputs["b2a"], dtype=F32)
    b2b = np.asarray(inputs["b2b"], dtype=F32)
    common = {
        "w1a": W1a.astype(BF16),
        "w1b": W1b.astype(BF16),
        "w2a_ap": W2a[:D].astype(BF16),
        "w2a_agg": W2a[D:].astype(BF16),
        "w2b": W2b.astype(BF16),
        "b2a": np.ascontiguousarray(b2a.reshape(H, 1)),
        "b2b": np.ascontiguousarray(b2b.reshape(D, 1)),
        "iota": np.ascontiguousarray(
            np.broadcast_to(np.arange(128, dtype=F32), (128, 128))
        ),
    }
    if has_b1a:
        b1a = np.asarray(inputs["b1a"], dtype=F32)
        common["b1a_mat"] = np.ascontiguousarray(np.broadcast_to(b1a, (128, H)))
    return common


def prepare(inputs):
    """Build everything needed to run: graph + per-core input maps."""
    b1a = np.asarray(inputs["b1a"], dtype=F32)
    b1b = np.asarray(inputs["b1b"], dtype=F32)
    has_b1a = bool(np.any(b1a != 0))
    has_b1b = bool(np.any(b1b != 0))

    per_core, T_blk, NTILE = _preprocess(inputs)
    nc = _build_graph(T_blk, has_b1a, has_b1b)

    ap_hid = np.asarray(inputs["ap_hid"], dtype=F32)
    common = _common_inputs(inputs, has_b1a)

    in_maps = []
    for c in range(NCORE):
        apT = np.zeros((D, NBLK * 128), dtype=BF16)
        apT[:, :APC] = ap_hid[c * APC : (c + 1) * APC].T.astype(BF16)
        m = dict(common)
        m["msgT"] = per_core[c]["msgT"]
        m["slots"] = per_core[c]["slots"]
        m["apT"] = apT
        if has_b1b:
            m["cntb1b"] = np.ascontiguousarray(
                (b1b[:, None] * per_core[c]["cnt"][:, None, :])
                .transpose(1, 0, 2).reshape(H, NBLK * 128)
            )
        in_maps.append(m)
    return nc, in_maps


def assemble_output(results):
    out = np.empty((N_AP, D), dtype=F32)
    for c in range(NCORE):
        out[c * APC : (c + 1) * APC] = results[c]["out"][:, :APC].T
    return out


def kernel(**inputs):
    nc, in_maps = prepare(inputs)
    res = run_bass_kernel_spmd(nc, in_maps, core_ids=list(range(NCORE)))
    return assemble_output(res.results)
